# revision 1
# baseline (speedup 1.0000x reference)
"""Trainium2 Bass kernel for nn_CrossAttentionNoGate.

Reference computation (per MSA row s):
    q = split_heads(x_q @ wq); k = split_heads(x_kv @ wk); v = split_heads(x_kv @ wv)
    a = softmax(q k^T/sqrt(D) + (mask-1)*INF + bias)
    out = merge_heads(a @ v) @ wo + bo

Sharding: S=128 rows split 16-per-core across 8 NeuronCores (data parallel);
weights and pair bias replicated.

Per-core design notes:
  - fp32 matmuls use float32r (single-pass reduced fp32); AV runs in bf16.
  - x^T comes from the 2-byte DMA-transpose xbar: x is split on the host into
    additive bf16 hi/lo halves, each [2 rows, 2C] pair-view block transposed,
    hi+lo summed into fp32 on GPSIMD.  The pair view permutes the sequence
    dims into (even, odd) halves; all other layouts account for that.
  - logits are computed transposed ([kv, q]) so the softmax-weight matrix
    feeds the AV matmul as the *moving* operand.
  - softmax without max-subtraction (logits are O(10) here): exp on ACT with
    the additive mask as per-partition activation bias; the pair bias enters
    multiplicatively: exp(l+b) = exp(l)*exp(b) with exp(bias^T) precomputed
    once on device (bf16), multiply split between DVE and GPSIMD.
  - PE array-tiling hazards: operands at mixed base partitions put the PE in
    tiled modes; concurrent tiles writing the same PSUM bank crash the
    device.  Hence: projections contract K=128 with zero-padded weights, the
    per-head QKT matmuls (bases 0/32/64) write bank-disjoint 512-strided
    column blocks, and AV uses 64-row padded v-columns at out bases {0, 64}.
  - AV appends a ones column (row 63/127 of each block = softmax denominator).
    Denominators are gathered to 8 partitions by tiny SBUF DMAs, inverted
    with reciprocal_approx_fast, and broadcast to a [128, 1024] R tile with a
    K=8 selector matmul on the PE; one elementwise multiply normalises.
  - output projection contracts the padded layout against wo_aug (zero rows
    kill the padding/denominator rows), giving [q, 64] natural layout that
    stores contiguously; the host unpermutes the q order.
"""

import math

import numpy as np

import concourse.bass as bass
import concourse.mybir as mybir
from concourse import bacc as _bacc
import concourse.tile as tile
from concourse import bass_utils

B, S, Q, KV = 1, 128, 256, 256
CQ, CKV = 64, 64
H, D = 8, 32
NCORES = 8
SC = S // NCORES
S2 = SC // 2
INF = 1.0e9
SCALE = 1.0 / math.sqrt(D)

F32 = mybir.dt.float32
F32R = mybir.dt.float32r
BF16 = mybir.dt.bfloat16
EXP = mybir.ActivationFunctionType.Exp


def _build(has_bo, reps=1):
    nc = _bacc.Bacc()

    xqh = nc.declare_dram_parameter("xqh", [S2, Q, 2 * CQ], BF16, isOutput=False)
    xql = nc.declare_dram_parameter("xql", [S2, Q, 2 * CQ], BF16, isOutput=False)
    xkh = nc.declare_dram_parameter("xkh", [S2, KV, 2 * CKV], BF16, isOutput=False)
    xkl = nc.declare_dram_parameter("xkl", [S2, KV, 2 * CKV], BF16, isOutput=False)
    biasT = nc.declare_dram_parameter("biasT", [2, 128, H * Q], F32, isOutput=False)
    maskcol = nc.declare_dram_parameter("maskcol", [128, SC, 2], F32, isOutput=False)
    esel = nc.declare_dram_parameter("esel", [8, 4, 128], F32R, isOutput=False)
    wq = nc.declare_dram_parameter("wq", [2, 128, H * D], F32R, isOutput=False)
    wk = nc.declare_dram_parameter("wk", [2, 128, H * D], F32R, isOutput=False)
    wv = nc.declare_dram_parameter("wv", [2, 128, H * D], F32R, isOutput=False)
    wo3 = nc.declare_dram_parameter("wo3", [4, 128, CQ], F32R, isOutput=False)
    if has_bo:
        bo1 = nc.declare_dram_parameter("bo1", [1, CQ], F32R, isOutput=False)
    out = nc.declare_dram_parameter("out", [SC, 128, 2 * CQ], F32R, isOutput=True)

    from contextlib import ExitStack

    with tile.TileContext(nc) as tc, ExitStack() as ctx:
        def pool(name, bufs, space="SBUF"):
            return ctx.enter_context(tc.tile_pool(name=name, bufs=bufs, space=space))

        singles = pool("singles", 1)
        stage = pool("stage", 1)
        xpp = pool("xpair", 32)
        xtp = pool("xt", 4)
        qkp = pool("qk", 3)
        vp = pool("vp", 3)
        expap = pool("expa", 10)
        expabp = pool("expab", 12)
        avsbp = pool("avsb", 3)
        otnp = pool("otn", 3)
        drp = pool("dr", 6)
        finp = pool("fin", 4)
        bigp = pool("bigp", 3, "PSUM")
        avp = pool("avp", 1, "PSUM")

        rep_cm = tc.For_i(0, reps, 1) if reps > 1 else None
        if rep_cm is not None:
            rep_cm.__enter__()
        if True:
            # ---- all xbar transposes first (the XPOSE struct has very few
            # sync-wait slots; nothing may precede them in DMA order)
            xp_tiles = []
            for s2 in range(S2):
                quad = []
                for hi_d, lo_d in ((xqh, xql), (xkh, xkl)):
                    th = xpp.tile([128, 256], BF16, tag="tph")
                    tl = xpp.tile([128, 256], BF16, tag="tpl")
                    nc.sync.dma_start(out=th[:], in_=hi_d[s2], transpose=True)
                    nc.sync.dma_start(out=tl[:], in_=lo_d[s2], transpose=True)
                    quad.append((th, tl))
                xp_tiles.append(quad)

            # ---- constants
            wq_sb = singles.tile([128, 2, H * D], F32R, tag="wq")
            wk_sb = singles.tile([128, 2, H * D], F32R, tag="wk")
            wv_sb = singles.tile([128, 2, H * D], F32R, tag="wv")
            wo_sb = singles.tile([128, 4 * CQ], F32R, tag="wo")
            for half in range(2):
                nc.sync.dma_start(out=wq_sb[:, half, :], in_=wq[half])
                nc.sync.dma_start(out=wk_sb[:, half, :], in_=wk[half])
                nc.sync.dma_start(out=wv_sb[:, half, :], in_=wv[half])
            for b4 in range(4):
                nc.sync.dma_start(out=wo_sb[:, CQ * b4 : CQ * (b4 + 1)], in_=wo3[b4])
            if has_bo:
                bo_sb = singles.tile([1, CQ], F32R, tag="bo")
                ones_sb = singles.tile([1, 128], F32R, tag="ones")
                nc.sync.dma_start(out=bo_sb[:], in_=bo1[:])
                nc.vector.memset(ones_sb[:], 1.0)

            mk_sb = singles.tile([128, SC, 2], F32, tag="mk")
            nc.sync.dma_start(out=mk_sb[:], in_=maskcol[:])
            esel_sb = singles.tile([8, 4, 128], F32R, tag="esel")
            nc.sync.dma_start(out=esel_sb[:], in_=esel[:])

            # exp(bias^T) in bf16: [kv-half(128), h*256 + qcol]
            expB = []
            for c in range(2):
                st = stage.tile([128, H * Q], F32, tag="bstage")
                nc.sync.dma_start(out=st[:], in_=biasT[c])
                eb = singles.tile([128, H * Q], BF16, tag=f"expB{c}")
                nc.scalar.activation(out=eb[:], in_=st[:], func=EXP)
                expB.append(eb)

            # ---- main loop over row pairs
            mm_idx = 0
            pending_tail = None
            for s2 in range(S2):
                # hi+lo merge of pre-transposed x^T halves (GPSIMD, all-SBUF)
                xparts = []
                for idx in range(2):
                    th, tl = xp_tiles[s2][idx]
                    xt = xtp.tile([128, 256], F32R, tag=f"xt{idx}")
                    nc.gpsimd.tensor_copy(xt[:], th[:])
                    nc.gpsimd.tensor_add(xt[:], xt[:], tl[:])
                    xparts.append(xt)
                xqT, xkvT = xparts

                # projections (all K=128, zero-padded weights).
                # qT/kT: 3 chunks of <=96 partitions (heads 3/3/2), cols
                # 512*c3 + 256*e + 128*sp + r, split into two psum tiles.
                # v: [128, 1024], cols 256*(2e+sp) + 32h + d.
                qT_a = bigp.tile([96, 1024], F32, tag="big")
                qT_b = bigp.tile([96, 512], F32, tag="big")
                kT_a = bigp.tile([96, 1024], F32, tag="big")
                kT_b = bigp.tile([96, 512], F32, tag="big")
                v_ps = bigp.tile([128, 1024], F32, tag="big")
                for c3 in range(3):
                    nh = 32 * (3 if c3 < 2 else 2)
                    for e in range(2):
                        if c3 < 2:
                            qdst = qT_a[0:nh, 512 * c3 + 256 * e : 512 * c3 + 256 * e + 256]
                            kdst = kT_a[0:nh, 512 * c3 + 256 * e : 512 * c3 + 256 * e + 256]
                        else:
                            qdst = qT_b[0:nh, 256 * e : 256 * e + 256]
                            kdst = kT_b[0:nh, 256 * e : 256 * e + 256]
                        nc.tensor.matmul(
                            qdst, wq_sb[:, e, 96 * c3 : 96 * c3 + nh], xqT[:]
                        )
                        nc.tensor.matmul(
                            kdst, wk_sb[:, e, 96 * c3 : 96 * c3 + nh], xkvT[:]
                        )
                for e in range(2):
                    for sp in range(2):
                        nc.tensor.matmul(
                            v_ps[:, 256 * (2 * e + sp) : 256 * (2 * e + sp) + 256],
                            xkvT[:, 128 * sp : 128 * sp + 128],
                            wv_sb[:, e, :],
                        )

                qT_sb = qkp.tile([96, 1536], F32R, tag="qT")
                kT_sb = qkp.tile([96, 1536], F32R, tag="kT")
                nc.scalar.copy(out=qT_sb[0:96, 0:1024], in_=qT_a[:])
                nc.scalar.copy(out=qT_sb[0:64, 1024:1536], in_=qT_b[0:64, :])
                nc.vector.tensor_copy(out=kT_sb[0:96, 0:1024], in_=kT_a[:])
                nc.vector.tensor_copy(out=kT_sb[0:64, 1024:1536], in_=kT_b[0:64, :])

                # v padded to 64 cols/head: [v(32) | zeros(31) | one]
                v_sb = vp.tile([128, 2, 2, H, 2 * D], BF16, tag="v")
                nc.vector.memset(v_sb[:, :, :, :, D : 2 * D - 1], 0.0)
                nc.vector.memset(v_sb[:, :, :, :, 2 * D - 1 : 2 * D], 1.0)
                for e in range(2):
                    for sp in range(2):
                        nc.vector.tensor_copy(
                            out=v_sb[:, sp, e, :, 0:D],
                            in_=v_ps[
                                :, 256 * (2 * e + sp) : 256 * (2 * e + sp) + 256
                            ].rearrange("p (h d) -> p h d", h=H),
                        )

                qv = qT_sb[:].rearrange(
                    "p (c3 e sp r) -> p c3 e sp r", e=2, sp=2, r=128
                )
                kv_ = kT_sb[:].rearrange(
                    "p (c3 e sp r) -> p c3 e sp r", e=2, sp=2, r=128
                )

                for sp in range(2):
                    s = 2 * s2 + sp
                    expabs = {}
                    for c in range(2):  # kv half
                        for t in range(4):  # head pair (2t, 2t+1)
                            # the two per-head matmuls run on different PE
                            # row-tiles concurrently: blocks at cols 0 / 512
                            # keep their banks disjoint
                            qkt = bigp.tile([128, 1024], F32, tag="big")
                            for u in range(2):
                                h = 2 * t + u
                                c3, g = h // 3, h % 3
                                nc.tensor.matmul(
                                    qkt[:, 512 * u : 512 * u + 256],
                                    kv_[32 * g : 32 * (g + 1), c3, c, sp, :],
                                    qv[32 * g : 32 * (g + 1), c3, :, sp, :],
                                )
                            expa = expap.tile([128, 512], BF16, tag="expa")
                            nc.scalar.activation(
                                out=expa[:].rearrange("p (u q) -> p u q", u=2),
                                in_=qkt[:].rearrange("p (u z) -> p u z", u=2)[
                                    :, :, 0:Q
                                ],
                                func=EXP,
                                bias=mk_sb[:, s, c : c + 1],
                            )
                            expab = expabp.tile([128, 512], BF16, tag="expab")
                            eng = nc.gpsimd if (mm_idx % 4 == 3) else nc.vector
                            eng.tensor_mul(
                                expab[:],
                                expa[:],
                                expB[c][:, 512 * t : 512 * (t + 1)],
                            )
                            mm_idx += 1
                            expabs[(c, t)] = expab

                    # late tail of the previous row: by now its inputs are
                    # a full iteration old, so these never stall the in-order
                    # engine queues (head-of-line blocking)
                    if pending_tail is not None:
                        pending_tail()
                        pending_tail = None

                    av_ps = avp.tile([128, 4 * Q], F32, tag="av")
                    # AV: kv halves back-to-back per head (psum accumulation
                    # groups must not interleave within a bank)
                    for t in range(4):
                        for u in range(2):
                            h = 2 * t + u
                            for c in range(2):
                                nc.tensor.matmul(
                                    av_ps[64 * u : 64 * u + 64, Q * t : Q * (t + 1)],
                                    v_sb[:, sp, c, h, :],
                                    expabs[(c, t)][:, Q * u : Q * (u + 1)],
                                    start=(c == 0),
                                    stop=(c == 1),
                                )

                    # AV psum -> SBUF (cols split ACT / DVE)
                    av_sb = avsbp.tile([128, 4 * Q], F32, tag="avsb")
                    nc.scalar.copy(out=av_sb[:, 0 : 2 * Q], in_=av_ps[:, 0 : 2 * Q])
                    nc.vector.tensor_copy(
                        out=av_sb[:, 2 * Q : 4 * Q], in_=av_ps[:, 2 * Q : 4 * Q]
                    )

                    # denominators (rows 63 / 127) -> 8 partitions
                    d_sb = drp.tile([H, Q], F32, tag="d")
                    for pi in range(2):
                        nc.sync.dma_start(
                            out=d_sb[4 * pi : 4 * pi + 4, :],
                            in_=av_sb[64 * pi + 63 : 64 * pi + 64, :],
                        )

                    def make_tail(s=s, d_sb=d_sb, av_sb=av_sb):
                        def tail():
                            r_sb = drp.tile([H, Q], F32, tag="r")
                            r_sr = drp.tile([H, Q], F32R, tag="rr")
                            nc.vector.reciprocal_approx_fast(
                                out=r_sb[:], in_=d_sb[:]
                            )
                            nc.vector.tensor_copy(out=r_sr[:], in_=r_sb[:])

                            # R[64*pi+d, Q*b+q] = r[2b+pi, q] via K=8
                            # selector matmuls (d rows in gather order 4pi+b)
                            R_ps = bigp.tile([128, 4 * Q], F32, tag="big")
                            for b4 in range(4):
                                nc.tensor.matmul(
                                    R_ps[:, Q * b4 : Q * (b4 + 1)],
                                    esel_sb[:, b4, :],
                                    r_sr[:],
                                )

                            otn = otnp.tile([128, 4 * Q], F32R, tag="otn")
                            nc.vector.tensor_mul(otn[:], av_sb[:], R_ps[:])

                            # output projection, natural [q, c] layout
                            fin_ps = bigp.tile([128, 2 * CQ], F32, tag="big")
                            for qc in range(2):
                                for b in range(4):
                                    nc.tensor.matmul(
                                        fin_ps[:, qc * CQ : (qc + 1) * CQ],
                                        otn[
                                            :,
                                            Q * b + 128 * qc : Q * b + 128 * qc + 128,
                                        ],
                                        wo_sb[:, CQ * b : CQ * (b + 1)],
                                        start=(b == 0),
                                        stop=(b == 3 and not has_bo),
                                    )
                                if has_bo:
                                    nc.tensor.matmul(
                                        fin_ps[:, qc * CQ : (qc + 1) * CQ],
                                        ones_sb[:],
                                        bo_sb[:],
                                        start=False,
                                        stop=True,
                                    )
                            fin_sb = finp.tile([128, 2 * CQ], F32R, tag="fin")
                            nc.vector.tensor_copy(out=fin_sb[:], in_=fin_ps[:])
                            nc.sync.dma_start(out=out[s], in_=fin_sb[:])
                        return tail

                    pending_tail = make_tail()

            if pending_tail is not None:
                pending_tail()
                pending_tail = None

        if rep_cm is not None:
            rep_cm.__exit__(None, None, None)
    nc.finalize()
    return nc


_CACHE = {}


def _get_nc(has_bo):
    if has_bo not in _CACHE:
        _CACHE[has_bo] = _build(has_bo)
    return _CACHE[has_bo]


def _host_prep(input_q, input_kv, mask, bias, wq, wk, wv, wo, bo):
    """Per-core input maps (host-side layout only)."""
    import ml_dtypes

    def zpad(w):  # [64, HD] -> [2, 128, HD], w on rows 64e..64e+63
        wz = np.zeros((2, 128, H * D), np.float32)
        wz[0, 0:64] = w
        wz[1, 64:128] = w
        return wz

    wq_s = zpad(wq.astype(np.float32) * SCALE)
    wk_s = zpad(wk.astype(np.float32))
    wv_s = zpad(wv.astype(np.float32))

    # bias^T, permuted: [c, kv-half row p (kv=2p+c), h*256 + qcol],
    # qcol = 128*(q%2) + q//2
    bt = bias[0, 0].astype(np.float32)  # [H, Q, KV]
    bt = bt.reshape(H, Q // 2, 2, KV // 2, 2)  # [h, qh, e, kvh, c]
    bt = np.ascontiguousarray(bt.transpose(4, 3, 0, 2, 1))  # [c, kvh, h, e, qh]
    biasT = bt.reshape(2, 128, H * Q)

    # additive mask, permuted kv: [p, s_local, c] with kv = 2p + c
    mterm = (mask[0, :, 0, 0, :].astype(np.float32) - 1.0) * INF  # [S, KV]
    mterm = mterm.reshape(S, KV // 2, 2)  # [s, p, c]

    # wo with padded-aug zero rows: wo_aug[h//2, 64*(h%2)+d] = wo[h*D+d]
    wo_aug = np.zeros((4, 128, CQ), np.float32)
    for h in range(H):
        wo_aug[h // 2, 64 * (h % 2) : 64 * (h % 2) + D] = wo[h * D : (h + 1) * D]

    # selector: esel[b4][k, m] = 1 iff k == 4*(m>=64) + b4
    # (d_sb rows are gather-order r = 4*pi + b)
    esel_h = np.zeros((8, 4, 128), np.float32)
    for b4 in range(4):
        esel_h[b4, b4, 0:64] = 1.0
        esel_h[4 + b4, b4, 64:128] = 1.0

    def split_pairs(x):  # [SC, L, C] fp32 -> hi/lo bf16 [SC//2, L, 2C]
        hi = x.astype(ml_dtypes.bfloat16)
        lo = (x - hi.astype(np.float32)).astype(ml_dtypes.bfloat16)
        n, L, C = x.shape
        return (
            np.ascontiguousarray(hi.reshape(n // 2, L, 2 * C)),
            np.ascontiguousarray(lo.reshape(n // 2, L, 2 * C)),
        )

    has_bo = bool(np.any(bo != 0))
    in_maps = []
    for i in range(NCORES):
        sl = slice(SC * i, SC * (i + 1))
        xq_h, xq_l = split_pairs(input_q[0, sl].astype(np.float32))
        xk_h, xk_l = split_pairs(input_kv[0, sl].astype(np.float32))
        m = {
            "xqh": xq_h,
            "xql": xq_l,
            "xkh": xk_h,
            "xkl": xk_l,
            "biasT": biasT,
            "maskcol": np.ascontiguousarray(mterm[sl].transpose(1, 0, 2)),
            "esel": esel_h,
            "wq": wq_s,
            "wk": wk_s,
            "wv": wv_s,
            "wo3": wo_aug,
        }
        if has_bo:
            m["bo1"] = np.ascontiguousarray(bo.astype(np.float32).reshape(1, CQ))
        in_maps.append(m)
    return has_bo, in_maps


def kernel(input_q, input_kv, mask, bias, wq, wk, wv, wo, bo, **_):
    has_bo, in_maps = _host_prep(input_q, input_kv, mask, bias, wq, wk, wv, wo, bo)
    nc = _get_nc(has_bo)
    res = bass_utils.run_bass_kernel_spmd(nc, in_maps, core_ids=list(range(NCORES)))
    outs = []
    for i in range(NCORES):
        o = res.results[i]["out"].reshape(SC, 128, 2, CQ)
        outs.append(o.reshape(SC, Q, CQ))  # q = 2p + qc flattens naturally
    full = np.concatenate(outs, axis=0).reshape(B, S, Q, CQ)
    return full.astype(np.float32)

